# revision 48
# baseline (speedup 1.0000x reference)
"""Trainium2 Bass kernel for a 2-layer encoder-decoder LSTM.

Problem: x [512, 256, 1] -> encoder 2-layer LSTM (H=512) -> autoregressive
decoder (64 steps, head feedback) -> out [512, 64].

Strategy: data-parallel across 8 NeuronCores (batch 512 -> 64 per core), all
weights replicated and SBUF-resident.  Per core each timestep needs 3
matmuls of [64,512] @ [512,2048], run "activation-stationary" (lhsT = h.T
chunk [128,64], moving = W.T [128,512] slices).  Each "pair" step computes
layer-1 @ t together with layer-0 @ t+1 so the PE always has a deep stream
of independent work.

Two dtype modes (LSTM_MMDT):
 - bfloat16: PE column tiling packs the two cells onto separate column
   groups (layer-1 -> PSUM partitions 0..63, layer-0 -> 64..127) which run
   concurrently, and the activation/vector engines process both cells
   stacked [128, *] at full width.
 - float32r: tf32-like full-width mode (the PE uses both columns of each
   column pair, so no column tiling is possible); the two cells use
   separate PSUM tiles on partitions 0..63.

Biases and the scalar input term are folded into extra K=1/K=2 matmul
accumulation passes.  h is re-transposed each step with PE transpose; the
transposed h.T doubles as the moving operand of the decoder head matmul.
"""

import os
import sys
import time

import numpy as np

B_FULL, T, H, HORIZON = 512, 256, 512, 64
NCORES = 8
B = B_FULL // NCORES          # 64 batch rows per core
G = 4 * H                     # 2048 gate columns
KC = H // 128                 # 4 contraction chunks
NCH = G // 512                # 4 output chunks of 512 (one PSUM bank each)

# gate chunk indices (PyTorch order: i, f, g, o)
I_SL, F_SL, G_SL, O_SL = 0, 1, 2, 3

_CACHE = {}
LAST_EXEC_NS = None
LAST_RESULTS = None


def _build(n_enc=T, n_dec=HORIZON, mm_dt="float32r"):
    """Build the Bass module (single SPMD program, run on all 8 cores)."""
    from contextlib import ExitStack

    import concourse.mybir as mybir
    import concourse.tile as tile
    from concourse import bacc
    from concourse.masks import make_identity

    dt = mybir.dt
    MDT = getattr(dt, mm_dt)
    F32 = dt.float32
    AF = mybir.ActivationFunctionType
    NT = n_enc + n_dec            # total timesteps
    PAIRED = mm_dt != "float32r"  # col-tiled two-group mode

    nc = bacc.Bacc("TRN2", target_bir_lowering=False, debug=False)

    # ---------------- DRAM parameters (per-core views) ----------------
    xaug = nc.declare_dram_parameter("xaug", [2, (n_enc + 1) * B], MDT, isOutput=False)
    wt = {}
    for nm in ("e0", "e1i", "e1h", "d0", "d1i", "d1h"):
        wt[nm] = nc.declare_dram_parameter(f"wt_{nm}", [128, KC, G], MDT, isOutput=False)
    rows_e0 = nc.declare_dram_parameter("rows_e0", [2, G], MDT, isOutput=False)
    rows_e1 = nc.declare_dram_parameter("rows_e1", [1, G], MDT, isOutput=False)
    rows_d0 = nc.declare_dram_parameter("rows_d0", [2, G], MDT, isOutput=False)
    rows_d1 = nc.declare_dram_parameter("rows_d1", [1, G], MDT, isOutput=False)
    headt_d = nc.declare_dram_parameter("headt", [128, KC], MDT, isOutput=False)
    headb_d = nc.declare_dram_parameter("headb", [1, B], MDT, isOutput=False)
    zeros_d = nc.declare_dram_parameter("zeros", [128, KC * B], MDT, isOutput=False)
    outT = nc.declare_dram_parameter("outT", [1, n_dec * B], F32, isOutput=True)

    with ExitStack() as ctx:
        tc = ctx.enter_context(tile.TileContext(nc))
        wpool = ctx.enter_context(tc.tile_pool(name="w", bufs=1))
        consts = ctx.enter_context(tc.tile_pool(name="consts", bufs=1))
        states = ctx.enter_context(tc.tile_pool(name="states", bufs=2))
        # loop-carried tags (fused state, c) need capacity >= 3 across the
        # For_i back edge and a buf count dividing the 8-pair body
        statesL = ctx.enter_context(tc.tile_pool(name="statesL", bufs=4))
        xstage = ctx.enter_context(tc.tile_pool(name="xstage", bufs=1))
        acts = ctx.enter_context(tc.tile_pool(name="acts", bufs=2))
        gpool = ctx.enter_context(
            tc.tile_pool(name="gp", bufs=4, space="PSUM"))
        tpool = ctx.enter_context(tc.tile_pool(name="tp", bufs=2, space="PSUM"))

        # ---------------- constants ----------------
        ident = consts.tile([128, 128], F32, tag="ident")
        make_identity(nc, ident)
        xall = consts.tile([2, (n_enc + 1) * B], MDT, tag="xall")
        nc.sync.dma_start(out=xall, in_=xaug[:, :])
        ones64 = consts.tile([1, B], MDT, tag="ones64")
        nc.sync.dma_start(out=ones64, in_=xaug[1:2, 0:B])
        headt = consts.tile([128, KC], MDT, tag="headt")
        nc.sync.dma_start(out=headt, in_=headt_d[:, :])
        headb = consts.tile([1, B], MDT, tag="headb")
        nc.sync.dma_start(out=headb, in_=headb_d[:, :])
        dec_stage = consts.tile([2, B], MDT, tag="dec_stage")
        # (x_last, ones): row 1 stays 1.0 forever; row 0 overwritten per step
        nc.vector.tensor_copy(dec_stage, xall[:, n_enc * B:(n_enc + 1) * B])
        out_acc = consts.tile([1, n_dec * B], F32, tag="out_acc")

        # weight tiles: encoder set now, decoder set later (same tags)
        def load_wset(phase):
            p = "e" if phase == 0 else "d"
            w0 = wpool.tile([128, KC, G], MDT, tag="w0")
            nc.sync.dma_start(out=w0, in_=wt[p + "0"][:, :, :])
            w1i = wpool.tile([128, KC, G], MDT, tag="w1i")
            nc.sync.dma_start(out=w1i, in_=wt[p + "1i"][:, :, :])
            w1h = wpool.tile([128, KC, G], MDT, tag="w1h")
            nc.sync.dma_start(out=w1h, in_=wt[p + "1h"][:, :, :])
            r0 = wpool.tile([2, G], MDT, tag="rows0")
            nc.sync.dma_start(out=r0, in_=(rows_e0 if phase == 0 else rows_d0)[:, :])
            r1 = wpool.tile([1, G], MDT, tag="rows1")
            nc.sync.dma_start(out=r1, in_=(rows_e1 if phase == 0 else rows_d1)[:, :])
            return dict(w0=w0, w1i=w1i, w1h=w1h, r0=r0, r1=r1)

        enc_w = load_wset(0)
        dec_w = None

        TOP = slice(0, 64)
        BOT = slice(64, 128)

        # ---------------- initial state ----------------
        # state accessors: h0ap(k)/h1ap(k) -> [128, B] lhsT chunk views
        h1T0 = states.tile([128, KC * B], MDT, tag="h1T")
        nc.sync.dma_start(out=h1T0, in_=zeros_d[:, :])
        h0ap = None                      # set by prologue
        h1ap = lambda k, t=h1T0: t[:, k * B:(k + 1) * B]
        if PAIRED:
            c_prev = statesL.tile([128, 512], F32, tag="c")
            nc.vector.memset(c_prev, 0.0)
            c1_prev = c0_prev = None
        else:
            c1_prev = states.tile([64, 512], F32, tag="c1")
            nc.vector.memset(c1_prev, 0.0)
            c0_prev = states.tile([64, 512], F32, tag="c0")
            nc.vector.memset(c0_prev, 0.0)
            c_prev = None

        def alloc_tset(sfx):
            return dict(
                ifsb=acts.tile([128, 1024], F32, tag="ifsb" + sfx, name="ifsb" + sfx),
                gsb=acts.tile([128, 512], F32, tag="gsb" + sfx, name="gsb" + sfx),
                osb=acts.tile([128, 512], F32, tag="osb" + sfx, name="osb" + sfx),
                t1=acts.tile([128, 512], F32, tag="t1" + sfx, name="t1" + sfx),
                t2=acts.tile([128, 512], F32, tag="t2" + sfx, name="t2" + sfx),
                tcsb=acts.tile([128, 512], F32, tag="tcsb" + sfx, name="tcsb" + sfx),
                hsb=acts.tile([128, 512], F32, tag="hsb" + sfx, name="hsb" + sfx),
            )

        def emit_cell(gps, gsl, ts, sl, c_prev_ap, c_new_ap):
            """One LSTM cell's activations + state update.
            gps: 4 psum chunk tiles; gsl: partition slice in psum;
            ts: act tile set; sl: partition slice in act tiles."""
            nc.scalar.activation(ts["ifsb"][sl, 0:512], gps[I_SL][gsl, :], AF.Sigmoid)
            nc.scalar.activation(ts["ifsb"][sl, 512:1024], gps[F_SL][gsl, :], AF.Sigmoid)
            nc.scalar.activation(ts["gsb"][sl, :], gps[G_SL][gsl, :], AF.Tanh)
            nc.vector.tensor_mul(ts["t1"][sl, :], ts["ifsb"][sl, 512:1024], c_prev_ap)
            nc.vector.tensor_mul(ts["t2"][sl, :], ts["ifsb"][sl, 0:512], ts["gsb"][sl, :])
            nc.vector.tensor_add(c_new_ap, ts["t1"][sl, :], ts["t2"][sl, :])
            # tanh(c) before sigmoid(o) in ACT program order: o depends on the
            # last-finishing gate chunk, tanh(c) only on i/f/g
            nc.scalar.activation(ts["tcsb"][sl, :], c_new_ap, AF.Tanh)
            nc.scalar.activation(ts["osb"][sl, :], gps[O_SL][gsl, :], AF.Sigmoid)
            nc.vector.tensor_mul(ts["hsb"][sl, :], ts["osb"][sl, :], ts["tcsb"][sl, :])

        def emit_transpose(h_src, ident_blk, state_tag):
            """h_src [64, 512] -> new [128, KC*B] transposed state tile."""
            tp = tpool.tile([128, KC * B], F32, tag="tp", name="tp" + state_tag)
            for k in range(KC):
                nc.tensor.transpose(tp[:, k * B:(k + 1) * B],
                                    h_src[:, k * 128:(k + 1) * 128], ident_blk)
            new = states.tile([128, KC * B], MDT, tag=state_tag, name=state_tag)
            nc.vector.tensor_copy(new, tp)
            return new

        def emit_transpose_fused(h_src):
            """h_src [128, 512] (both cells) -> [128, 512] fused state tile.
            Chunk k cols 0:64 = TOP cell h.T, cols 64:128 = BOT cell h.T."""
            tps = []
            for half in range(2):
                tp = tpool.tile([128, KC * B], F32, tag="tp", name=f"tpf{half}")
                for kk in range(2):
                    k = half * 2 + kk
                    nc.tensor.transpose(tp[:, kk * 128:(kk + 1) * 128],
                                        h_src[:, k * 128:(k + 1) * 128], ident)
                tps.append(tp)
            new = statesL.tile([128, 512], MDT, tag="hTf", name="hTf")
            nc.vector.tensor_copy(new[:, 0:256], tps[0])
            nc.vector.tensor_copy(new[:, 256:512], tps[1])
            return new

        def emit_pair(s, top, bottom, stage_override=None):
            """TOP: layer-1 cell @ time s.  BOTTOM: layer-0 cell @ time s+1."""
            nonlocal h0ap, h1ap, c_prev, c1_prev, c0_prev, dec_w
            u = s + 1  # bottom timestep
            if bottom and u >= n_enc and dec_w is None:
                dec_w = load_wset(1)
            wtop = enc_w if (top and s < n_enc) else dec_w
            wbot = enc_w if (bottom and u < n_enc) else dec_w
            feedback = bottom and u > n_enc  # bottom x comes from this pair's head

            mm_h0ap, mm_h1ap = h0ap, h1ap
            stage = None
            if bottom:
                if stage_override is not None:
                    stage = stage_override
                elif not feedback:  # encoder steps + first decoder step: resident
                    stage = xall[:, u * B:(u + 1) * B]
                else:
                    stage = dec_stage

            # ---------------- matmul passes ----------------
            if PAIRED:
                gps_t = gps_b = [gpool.tile([128, 512], F32, tag="gp", name=f"gp{j}")
                                 for j in range(NCH)]
                bot_gsl, bot_tpos = BOT, (0, 64)
            else:
                gps_t = [gpool.tile([64, 512], F32, tag="gp", name=f"gpt{j}")
                         for j in range(NCH)] if top else None
                gps_b = [gpool.tile([64, 512], F32, tag="gp", name=f"gpb{j}")
                         for j in range(NCH)] if bottom else None
                bot_gsl, bot_tpos = slice(0, 64), (0, 0)

            a_seq = []  # top: bias1, wih1 x16, whh1 x16
            b_seq = []  # bottom: whh0 x16 (+ xb x4 if not feedback)
            first_b = [True] * NCH
            if top:
                for j in range(NCH):
                    a_seq.append((gps_t[j][TOP, :], ones64[0:1, :],
                                  wtop["r1"][0:1, j * 512:(j + 1) * 512], True, False))
                for j in range(NCH):
                    for k in range(KC):
                        a_seq.append((gps_t[j][TOP, :], mm_h0ap(k),
                                      wtop["w1i"][:, k, j * 512:(j + 1) * 512], False, False))
                # chunk-contiguous: chunk j's gates complete in order i,f,g,o so
                # the ACT/cell chain starts while later chunks still accumulate
                for j in range(NCH):
                    for k in range(KC):
                        a_seq.append((gps_t[j][TOP, :], mm_h1ap(k),
                                      wtop["w1h"][:, k, j * 512:(j + 1) * 512],
                                      False, k == KC - 1))
            if bottom:
                for j in range(NCH):
                    if mm_h0ap is not None:
                        for k in range(KC):
                            b_seq.append((gps_b[j][bot_gsl, :], mm_h0ap(k),
                                          wbot["w0"][:, k, j * 512:(j + 1) * 512],
                                          first_b[j], False))
                            first_b[j] = False
                    if not feedback:
                        b_seq.append((gps_b[j][bot_gsl, :], stage[0:2, :],
                                      wbot["r0"][0:2, j * 512:(j + 1) * 512],
                                      first_b[j], True))
                        first_b[j] = False

            # emission order: bias passes, then bottom-dense 1:1 with top, then
            # the rest of top.  The bottom cell's matmuls finish mid-pair so its
            # h.T (needed by almost all of the next pair) is ready by pair end.
            # emit_mms(phase=0) emits through the end of the bottom stream (the
            # caller then emits the bottom cell + transposes so they land
            # mid-stream in the PE queue); emit_mms(phase=1) emits the rest.
            nbias = NCH if top else 0
            na, nb = len(a_seq), len(b_seq)
            order = [("a", x) for x in a_seq[:nbias]]
            ia, ib = nbias, 0
            if PAIRED:
                # 1:1 zip: both column groups advance at their own full rate
                # (starts are pc-monotone but execution is concurrent), so the
                # bottom group finishes at ~nb passes while A streams on
                while ib < nb:
                    order.append(("b", b_seq[ib])); ib += 1
                    if ia < na:
                        order.append(("a", a_seq[ia])); ia += 1
            else:
                order.extend(("b", x) for x in b_seq)  # bottom block first, dense
                ib = nb
            split0 = len(order)
            order.extend(("a", x) for x in a_seq[ia:])
            # phase boundaries: [0: bias+bottom][1: ~12 top passes][2: rest]
            split1 = min(split0 + 12, len(order))

            def emit_mms(phase):
                lo, hi = [(0, split0), (split0, split1), (split1, len(order))][phase]
                for grp, (out, lhsT, rhs, st, sp) in order[lo:hi]:
                    nc.tensor.matmul(out, lhsT, rhs, start=st, stop=sp,
                                     tile_position=(0, 0) if grp == "a" else bot_tpos,
                                     skip_group_check=True)

            # ---------------- activations + cell + transpose ----------------
            if PAIRED:
                ts_t = ts_b = alloc_tset("")
                c_new = statesL.tile([128, 512], F32, tag="c", name="c")
                if not (top and bottom):
                    nc.vector.memset(c_new[BOT if top else TOP, :], 0.0)
                cell_top = lambda: emit_cell(gps_t, TOP, ts_t, TOP,
                                             c_prev[TOP, :], c_new[TOP, :])
                cell_bot = lambda: emit_cell(gps_b, BOT, ts_b, BOT,
                                             c_prev[BOT, :], c_new[BOT, :])
                top_h = lambda: ts_t["hsb"][TOP, :]
                bot_h = lambda: ts_b["hsb"][BOT, :]
                bot_ident = ident[64:128, 64:128]
            else:
                ts_t = alloc_tset("t") if top else None
                ts_b = alloc_tset("b") if bottom else None
                c1_new = (states.tile([64, 512], F32, tag="c1", name="c1")
                          if top else None)
                c0_new = (states.tile([64, 512], F32, tag="c0", name="c0")
                          if bottom else None)
                cell_top = lambda: emit_cell(gps_t, slice(0, 64), ts_t, TOP,
                                             c1_prev[:, :], c1_new[:, :])
                cell_bot = lambda: emit_cell(gps_b, slice(0, 64), ts_b, TOP,
                                             c0_prev[:, :], c0_new[:, :])
                top_h = lambda: ts_t["hsb"][TOP, :]
                bot_h = lambda: ts_b["hsb"][TOP, :]
                bot_ident = ident[0:64, 0:64]

            def head():
                d = s - n_enc
                hd = tpool.tile([128, KC * B], F32, tag="tp", name="hd")[0:1, 0:B]
                nc.tensor.matmul(hd, ones64[0:1, 0:1], headb[0:1, :],
                                 start=True, stop=False)
                for k in range(KC):
                    nc.tensor.matmul(hd, headt[:, k:k + 1], h1ap(k),
                                     start=False, stop=k == KC - 1)
                if d + 1 < n_dec:
                    nc.vector.tensor_copy(dec_stage[0:1, :], hd)
                nc.vector.tensor_copy(out_acc[0:1, d * B:(d + 1) * B], hd)

            if feedback:
                # decoder: top cell -> head -> bottom x pass -> bottom cell
                emit_mms(0)
                emit_mms(1)
                emit_mms(2)
                cell_top()
                t1T = emit_transpose(top_h(), ident[0:64, 0:64], "h1T")
                h1ap = lambda k, t=t1T: t[:, k * B:(k + 1) * B]
                head()
                for j in range(NCH):
                    nc.tensor.matmul(gps_b[j][bot_gsl, :], stage[0:2, :],
                                     wbot["r0"][0:2, j * 512:(j + 1) * 512],
                                     start=first_b[j], stop=True,
                                     tile_position=bot_tpos, skip_group_check=True)
                cell_bot()
                t0T = emit_transpose(bot_h(), bot_ident, "h0T")
                h0ap = lambda k, t=t0T: t[:, k * B:(k + 1) * B]
            elif PAIRED and top and bottom:
                # full-width path: one cell chain for both cells + fused
                # transposes (half the ACT/DVE instructions)
                emit_mms(0)
                emit_mms(1)
                emit_mms(2)
                emit_cell(gps_t, slice(0, 128), ts_t, slice(0, 128),
                          c_prev[:, :], c_new[:, :])
                fused = emit_transpose_fused(ts_t["hsb"])
                h1ap = lambda k, t=fused: t[:, k * 128:k * 128 + 64]
                h0ap = lambda k, t=fused: t[:, k * 128 + 64:(k + 1) * 128]
            else:
                # bottom first; its transposes go into the PE stream a dozen
                # passes later so the PE reaches them just as the bottom
                # cell's ACT/DVE chain finishes (no head-of-line stall)
                emit_mms(0)
                if bottom:
                    cell_bot()
                emit_mms(1)
                if bottom:
                    t0T = emit_transpose(bot_h(), bot_ident, "h0T")
                    h0ap = lambda k, t=t0T: t[:, k * B:(k + 1) * B]
                emit_mms(2)
                if top:
                    cell_top()
                    t1T = emit_transpose(top_h(), ident[0:64, 0:64], "h1T")
                    h1ap = lambda k, t=t1T: t[:, k * B:(k + 1) * B]
                    if s >= n_enc:
                        head()

            if PAIRED:
                c_prev = c_new
            else:
                if top:
                    c1_prev = c1_new
                if bottom:
                    c0_prev = c0_new

        # prologue: layer-0 @ t=0 alone, then first fw pair unrolled
        emit_pair(-1, top=False, bottom=True)
        BODY = 8
        # encoder fw pairs s=1..n_enc-2 go through a hardware loop (8-pair
        # body); everything s-dependent inside is either constant for these
        # pairs (enc weights, no head) or overridden (x stage).
        nloop = max(0, (n_enc - 2) // BODY) if PAIRED else 0
        s_after_loop = 1 + nloop * BODY
        emit_pair(0, top=True, bottom=True)
        if nloop > 0:
            from concourse.bass import ds
            with tc.For_i(0, nloop, 1) as iv:
                stage_body = xstage.tile([2, BODY * B], MDT, tag="stage_body")
                nc.sync.dma_start(
                    out=stage_body,
                    in_=xaug[:, ds(2 * B + iv * (BODY * B), BODY * B)])
                for p in range(BODY):
                    emit_pair(1 + p, top=True, bottom=True,
                              stage_override=stage_body[:, p * B:(p + 1) * B])
        for s in range(s_after_loop, NT - 1):
            emit_pair(s, top=True, bottom=True)
        emit_pair(NT - 1, top=True, bottom=False)
        nc.sync.dma_start(out=outT[:, :], in_=out_acc)

    nc.compile()
    return nc


# ------------------------------------------------------------------
# host-side packing
# ------------------------------------------------------------------
def _np_dt(mm_dt):
    if mm_dt == "bfloat16":
        import ml_dtypes
        return ml_dtypes.bfloat16
    if mm_dt == "float16":
        return np.float16
    return np.float32


def _pack_weights(inputs, mm_dt="float32r"):
    f32 = np.float32
    ndt = _np_dt(mm_dt)

    def wt_pack(w):  # [G, H] -> [128, KC, G]
        return np.ascontiguousarray(
            np.asarray(w, f32).T.reshape(KC, 128, G).transpose(1, 0, 2)).astype(ndt)

    m = {
        "wt_e0": wt_pack(inputs["enc_Whh0"]),
        "wt_e1i": wt_pack(inputs["enc_Wih1"]),
        "wt_e1h": wt_pack(inputs["enc_Whh1"]),
        "wt_d0": wt_pack(inputs["dec_Whh0"]),
        "wt_d1i": wt_pack(inputs["dec_Wih1"]),
        "wt_d1h": wt_pack(inputs["dec_Whh1"]),
        "rows_e0": np.stack([np.asarray(inputs["enc_Wih0"], f32)[:, 0],
                             np.asarray(inputs["enc_b0"], f32)]).astype(ndt),
        "rows_e1": np.asarray(inputs["enc_b1"], f32)[None, :].astype(ndt),
        "rows_d0": np.stack([np.asarray(inputs["dec_Wih0"], f32)[:, 0],
                             np.asarray(inputs["dec_b0"], f32)]).astype(ndt),
        "rows_d1": np.asarray(inputs["dec_b1"], f32)[None, :].astype(ndt),
        "headt": np.ascontiguousarray(
            np.asarray(inputs["head_W"], f32)[0].reshape(KC, 128).T).astype(ndt),
        "headb": np.full((1, B), float(np.asarray(inputs["head_b"])[0]), ndt),
        "zeros": np.zeros((128, KC * B), ndt),
    }
    return {k: np.ascontiguousarray(v) for k, v in m.items()}


def _pack_x(xc, n_enc=T, mm_dt="float32r"):
    """xc [B, T, 1] slice -> xaug [2, (n_enc+1)*B] (row0 = x_t seq, row1 = 1)."""
    f32 = np.float32
    xt = np.asarray(xc, f32)[:, :, 0].T  # [T, B]
    xa = np.empty((2, (n_enc + 1) * B), f32)
    xa[0, :n_enc * B] = xt[:n_enc].reshape(-1)
    xa[0, n_enc * B:] = xt[T - 1]  # decoder initial input = last observed x
    xa[1, :] = 1.0
    return np.ascontiguousarray(xa.astype(_np_dt(mm_dt)))


def kernel(**inputs):
    global LAST_EXEC_NS, LAST_RESULTS
    from concourse.bass_utils import run_bass_kernel_spmd

    n_enc = int(os.environ.get("LSTM_NENC", T))
    n_dec = int(os.environ.get("LSTM_NDEC", HORIZON))
    mm_dt = os.environ.get("LSTM_MMDT", "float16")
    key = (n_enc, n_dec, mm_dt)
    if key not in _CACHE:
        _CACHE[key] = _build(n_enc, n_dec, mm_dt)
    nc = _CACHE[key]

    shared = _pack_weights(inputs, mm_dt)
    in_maps = []
    for c in range(NCORES):
        m = dict(shared)
        m["xaug"] = _pack_x(inputs["x"][c * B:(c + 1) * B], n_enc, mm_dt)
        in_maps.append(m)

    trace = os.environ.get("LSTM_TRACE", "0") == "1"
    t0 = time.time()
    res = run_bass_kernel_spmd(nc, in_maps, list(range(NCORES)), trace=trace)
    wall = time.time() - t0
    LAST_EXEC_NS = res.exec_time_ns
    LAST_RESULTS = res
    if os.environ.get("LSTM_VERBOSE", "0") == "1":
        print(f"[kernel] wall={wall:.2f}s exec_time_ns={res.exec_time_ns}", file=sys.stderr)

    out = np.empty((B_FULL, n_dec), np.float32)
    for c in range(NCORES):
        out[c * B:(c + 1) * B, :] = res.results[c]["outT"].reshape(n_dec, B).T
    return out


if __name__ == "__main__":
    cmd = sys.argv[1] if len(sys.argv) > 1 else "build"
    if cmd == "build":
        ne = int(os.environ.get("LSTM_NENC", "4"))
        nd = int(os.environ.get("LSTM_NDEC", "2"))
        md = os.environ.get("LSTM_MMDT", "float32r")
        t0 = time.time()
        nc = _build(ne, nd, md)
        print(f"build({ne},{nd},{md}) ok in {time.time()-t0:.1f}s")



# revision 52
# speedup vs baseline: 1.0480x; 1.0480x over previous
"""Trainium2 Bass kernel for a 2-layer encoder-decoder LSTM.

Problem: x [512, 256, 1] -> encoder 2-layer LSTM (H=512) -> autoregressive
decoder (64 steps, head feedback) -> out [512, 64].

Strategy: data-parallel across 8 NeuronCores (batch 512 -> 64 per core), all
weights replicated and SBUF-resident.  Per core each timestep needs 3
matmuls of [64,512] @ [512,2048], run "activation-stationary" (lhsT = h.T
chunk [128,64], moving = W.T [128,512] slices).  Each "pair" step computes
layer-1 @ t together with layer-0 @ t+1 so the PE always has a deep stream
of independent work.

Two dtype modes (LSTM_MMDT):
 - bfloat16: PE column tiling packs the two cells onto separate column
   groups (layer-1 -> PSUM partitions 0..63, layer-0 -> 64..127) which run
   concurrently, and the activation/vector engines process both cells
   stacked [128, *] at full width.
 - float32r: tf32-like full-width mode (the PE uses both columns of each
   column pair, so no column tiling is possible); the two cells use
   separate PSUM tiles on partitions 0..63.

Biases and the scalar input term are folded into extra K=1/K=2 matmul
accumulation passes.  h is re-transposed each step with PE transpose; the
transposed h.T doubles as the moving operand of the decoder head matmul.
"""

import os
import sys
import time

import numpy as np

B_FULL, T, H, HORIZON = 512, 256, 512, 64
NCORES = 8
B = B_FULL // NCORES          # 64 batch rows per core
G = 4 * H                     # 2048 gate columns
KC = H // 128                 # 4 contraction chunks
NCH = G // 512                # 4 output chunks of 512 (one PSUM bank each)

# gate chunk indices (PyTorch order: i, f, g, o)
I_SL, F_SL, G_SL, O_SL = 0, 1, 2, 3

_CACHE = {}
LAST_EXEC_NS = None
LAST_RESULTS = None


def _build(n_enc=T, n_dec=HORIZON, mm_dt="float32r"):
    """Build the Bass module (single SPMD program, run on all 8 cores)."""
    from contextlib import ExitStack

    import concourse.mybir as mybir
    import concourse.tile as tile
    from concourse import bacc
    from concourse.masks import make_identity

    dt = mybir.dt
    MDT = getattr(dt, mm_dt)
    F32 = dt.float32
    AF = mybir.ActivationFunctionType
    NT = n_enc + n_dec            # total timesteps
    PAIRED = mm_dt != "float32r"  # col-tiled two-group mode

    nc = bacc.Bacc("TRN2", target_bir_lowering=False, debug=False)

    # ---------------- DRAM parameters (per-core views) ----------------
    xaug = nc.declare_dram_parameter("xaug", [2, (n_enc + 1) * B], MDT, isOutput=False)
    wt = {}
    for nm in ("e0", "e1i", "e1h", "d0", "d1i", "d1h"):
        wt[nm] = nc.declare_dram_parameter(f"wt_{nm}", [128, KC, G], MDT, isOutput=False)
    rows_e0 = nc.declare_dram_parameter("rows_e0", [2, G], MDT, isOutput=False)
    rows_e1 = nc.declare_dram_parameter("rows_e1", [1, G], MDT, isOutput=False)
    rows_d0 = nc.declare_dram_parameter("rows_d0", [2, G], MDT, isOutput=False)
    rows_d1 = nc.declare_dram_parameter("rows_d1", [1, G], MDT, isOutput=False)
    headt_d = nc.declare_dram_parameter("headt", [128, KC], MDT, isOutput=False)
    headb_d = nc.declare_dram_parameter("headb", [1, B], MDT, isOutput=False)
    zeros_d = nc.declare_dram_parameter("zeros", [128, KC * B], MDT, isOutput=False)
    outT = nc.declare_dram_parameter("outT", [1, n_dec * B], F32, isOutput=True)

    with ExitStack() as ctx:
        tc = ctx.enter_context(tile.TileContext(nc))
        wpool = ctx.enter_context(tc.tile_pool(name="w", bufs=1))
        consts = ctx.enter_context(tc.tile_pool(name="consts", bufs=1))
        states = ctx.enter_context(tc.tile_pool(name="states", bufs=2))
        # loop-carried tags (fused state, c) need capacity >= 3 across the
        # For_i back edge and a buf count dividing the 8-pair body
        statesL = ctx.enter_context(tc.tile_pool(name="statesL", bufs=4))
        xstage = ctx.enter_context(tc.tile_pool(name="xstage", bufs=1))
        acts = ctx.enter_context(tc.tile_pool(name="acts", bufs=2))
        gpool = ctx.enter_context(
            tc.tile_pool(name="gp", bufs=4, space="PSUM"))
        tpool = ctx.enter_context(tc.tile_pool(name="tp", bufs=2, space="PSUM"))

        # ---------------- constants ----------------
        ident = consts.tile([128, 128], F32, tag="ident")
        make_identity(nc, ident)
        identM = consts.tile([128, 128], MDT, tag="identM")
        make_identity(nc, identM)
        xall = consts.tile([2, (n_enc + 1) * B], MDT, tag="xall")
        nc.sync.dma_start(out=xall, in_=xaug[:, :])
        ones64 = consts.tile([1, B], MDT, tag="ones64")
        nc.sync.dma_start(out=ones64, in_=xaug[1:2, 0:B])
        headt = consts.tile([128, KC], MDT, tag="headt")
        nc.sync.dma_start(out=headt, in_=headt_d[:, :])
        headb = consts.tile([1, B], MDT, tag="headb")
        nc.sync.dma_start(out=headb, in_=headb_d[:, :])
        dec_stage = consts.tile([2, B], MDT, tag="dec_stage")
        # (x_last, ones): row 1 stays 1.0 forever; row 0 overwritten per step
        nc.vector.tensor_copy(dec_stage, xall[:, n_enc * B:(n_enc + 1) * B])
        out_acc = consts.tile([1, n_dec * B], F32, tag="out_acc")

        # weight tiles: encoder set now, decoder set later (same tags)
        def load_wset(phase):
            p = "e" if phase == 0 else "d"
            w0 = wpool.tile([128, KC, G], MDT, tag="w0")
            nc.sync.dma_start(out=w0, in_=wt[p + "0"][:, :, :])
            w1i = wpool.tile([128, KC, G], MDT, tag="w1i")
            nc.sync.dma_start(out=w1i, in_=wt[p + "1i"][:, :, :])
            w1h = wpool.tile([128, KC, G], MDT, tag="w1h")
            nc.sync.dma_start(out=w1h, in_=wt[p + "1h"][:, :, :])
            r0 = wpool.tile([2, G], MDT, tag="rows0")
            nc.sync.dma_start(out=r0, in_=(rows_e0 if phase == 0 else rows_d0)[:, :])
            r1 = wpool.tile([1, G], MDT, tag="rows1")
            nc.sync.dma_start(out=r1, in_=(rows_e1 if phase == 0 else rows_d1)[:, :])
            return dict(w0=w0, w1i=w1i, w1h=w1h, r0=r0, r1=r1)

        enc_w = load_wset(0)
        dec_w = None

        TOP = slice(0, 64)
        BOT = slice(64, 128)

        # ---------------- initial state ----------------
        # state accessors: h0ap(k)/h1ap(k) -> [128, B] lhsT chunk views
        h1T0 = states.tile([128, KC * B], MDT, tag="h1T")
        nc.sync.dma_start(out=h1T0, in_=zeros_d[:, :])
        h0ap = None                      # set by prologue
        h1ap = lambda k, t=h1T0: t[:, k * B:(k + 1) * B]
        if PAIRED:
            c_prev = statesL.tile([128, 512], F32, tag="c")
            nc.vector.memset(c_prev, 0.0)
            c1_prev = c0_prev = None
        else:
            c1_prev = states.tile([64, 512], F32, tag="c1")
            nc.vector.memset(c1_prev, 0.0)
            c0_prev = states.tile([64, 512], F32, tag="c0")
            nc.vector.memset(c0_prev, 0.0)
            c_prev = None

        def alloc_tset(sfx):
            return dict(
                ifsb=acts.tile([128, 1024], F32, tag="ifsb" + sfx, name="ifsb" + sfx),
                gsb=acts.tile([128, 512], F32, tag="gsb" + sfx, name="gsb" + sfx),
                osb=acts.tile([128, 512], F32, tag="osb" + sfx, name="osb" + sfx),
                t1=acts.tile([128, 512], F32, tag="t1" + sfx, name="t1" + sfx),
                t2=acts.tile([128, 512], F32, tag="t2" + sfx, name="t2" + sfx),
                tcsb=acts.tile([128, 512], F32, tag="tcsb" + sfx, name="tcsb" + sfx),
                # MDT: h is fp16 in the state anyway; fp16 input makes the PE
                # transpose run at 1 cycle/row instead of 2
                hsb=acts.tile([128, 512], MDT, tag="hsb" + sfx, name="hsb" + sfx),
            )

        def emit_cell(gps, gsl, ts, sl, c_prev_ap, c_new_ap):
            """One LSTM cell's activations + state update.
            gps: 4 psum chunk tiles; gsl: partition slice in psum;
            ts: act tile set; sl: partition slice in act tiles."""
            nc.scalar.activation(ts["ifsb"][sl, 0:512], gps[I_SL][gsl, :], AF.Sigmoid)
            nc.scalar.activation(ts["ifsb"][sl, 512:1024], gps[F_SL][gsl, :], AF.Sigmoid)
            nc.scalar.activation(ts["gsb"][sl, :], gps[G_SL][gsl, :], AF.Tanh)
            nc.vector.tensor_mul(ts["t1"][sl, :], ts["ifsb"][sl, 512:1024], c_prev_ap)
            nc.vector.tensor_mul(ts["t2"][sl, :], ts["ifsb"][sl, 0:512], ts["gsb"][sl, :])
            nc.vector.tensor_add(c_new_ap, ts["t1"][sl, :], ts["t2"][sl, :])
            # tanh(c) before sigmoid(o) in ACT program order: o depends on the
            # last-finishing gate chunk, tanh(c) only on i/f/g
            nc.scalar.activation(ts["tcsb"][sl, :], c_new_ap, AF.Tanh)
            nc.scalar.activation(ts["osb"][sl, :], gps[O_SL][gsl, :], AF.Sigmoid)
            nc.vector.tensor_mul(ts["hsb"][sl, :], ts["osb"][sl, :], ts["tcsb"][sl, :])

        def emit_transpose(h_src, ident_blk, state_tag):
            """h_src [64, 512] -> new [128, KC*B] transposed state tile."""
            tp = tpool.tile([128, KC * B], MDT, tag="tp", name="tp" + state_tag)
            for k in range(KC):
                nc.tensor.transpose(tp[:, k * B:(k + 1) * B],
                                    h_src[:, k * 128:(k + 1) * 128], ident_blk)
            new = states.tile([128, KC * B], MDT, tag=state_tag, name=state_tag)
            nc.vector.tensor_copy(new, tp)
            return new

        def emit_transpose_fused(h_src):
            """h_src [128, 512] (both cells) -> [128, 512] fused state tile.
            Chunk k cols 0:64 = TOP cell h.T, cols 64:128 = BOT cell h.T."""
            tps = []
            for half in range(2):
                tp = tpool.tile([128, KC * B], F32, tag="tp", name=f"tpf{half}")
                for kk in range(2):
                    k = half * 2 + kk
                    nc.tensor.transpose(tp[:, kk * 128:(kk + 1) * 128],
                                        h_src[:, k * 128:(k + 1) * 128], ident)
                tps.append(tp)
            new = statesL.tile([128, 512], MDT, tag="hTf", name="hTf")
            nc.vector.tensor_copy(new[:, 0:256], tps[0])
            nc.vector.tensor_copy(new[:, 256:512], tps[1])
            return new

        def emit_pair(s, top, bottom, stage_override=None):
            """TOP: layer-1 cell @ time s.  BOTTOM: layer-0 cell @ time s+1."""
            nonlocal h0ap, h1ap, c_prev, c1_prev, c0_prev, dec_w
            u = s + 1  # bottom timestep
            if bottom and u >= n_enc and dec_w is None:
                dec_w = load_wset(1)
            wtop = enc_w if (top and s < n_enc) else dec_w
            wbot = enc_w if (bottom and u < n_enc) else dec_w
            feedback = bottom and u > n_enc  # bottom x comes from this pair's head

            mm_h0ap, mm_h1ap = h0ap, h1ap
            stage = None
            if bottom:
                if stage_override is not None:
                    stage = stage_override
                elif not feedback:  # encoder steps + first decoder step: resident
                    stage = xall[:, u * B:(u + 1) * B]
                else:
                    stage = dec_stage

            # ---------------- matmul passes ----------------
            if PAIRED:
                gps_t = gps_b = [gpool.tile([128, 512], F32, tag="gp", name=f"gp{j}")
                                 for j in range(NCH)]
                bot_gsl, bot_tpos = BOT, (0, 64)
            else:
                gps_t = [gpool.tile([64, 512], F32, tag="gp", name=f"gpt{j}")
                         for j in range(NCH)] if top else None
                gps_b = [gpool.tile([64, 512], F32, tag="gp", name=f"gpb{j}")
                         for j in range(NCH)] if bottom else None
                bot_gsl, bot_tpos = slice(0, 64), (0, 0)

            a_seq = []  # top: bias1, wih1 x16, whh1 x16
            b_seq = []  # bottom: whh0 x16 (+ xb x4 if not feedback)
            first_b = [True] * NCH
            if top:
                for j in range(NCH):
                    a_seq.append((gps_t[j][TOP, :], ones64[0:1, :],
                                  wtop["r1"][0:1, j * 512:(j + 1) * 512], True, False))
                for j in range(NCH):
                    for k in range(KC):
                        a_seq.append((gps_t[j][TOP, :], mm_h0ap(k),
                                      wtop["w1i"][:, k, j * 512:(j + 1) * 512], False, False))
                # chunk-contiguous: chunk j's gates complete in order i,f,g,o so
                # the ACT/cell chain starts while later chunks still accumulate
                for j in range(NCH):
                    for k in range(KC):
                        a_seq.append((gps_t[j][TOP, :], mm_h1ap(k),
                                      wtop["w1h"][:, k, j * 512:(j + 1) * 512],
                                      False, k == KC - 1))
            if bottom:
                for j in range(NCH):
                    if mm_h0ap is not None:
                        for k in range(KC):
                            b_seq.append((gps_b[j][bot_gsl, :], mm_h0ap(k),
                                          wbot["w0"][:, k, j * 512:(j + 1) * 512],
                                          first_b[j], False))
                            first_b[j] = False
                    if not feedback:
                        b_seq.append((gps_b[j][bot_gsl, :], stage[0:2, :],
                                      wbot["r0"][0:2, j * 512:(j + 1) * 512],
                                      first_b[j], True))
                        first_b[j] = False

            # emission order: bias passes, then bottom-dense 1:1 with top, then
            # the rest of top.  The bottom cell's matmuls finish mid-pair so its
            # h.T (needed by almost all of the next pair) is ready by pair end.
            # emit_mms(phase=0) emits through the end of the bottom stream (the
            # caller then emits the bottom cell + transposes so they land
            # mid-stream in the PE queue); emit_mms(phase=1) emits the rest.
            nbias = NCH if top else 0
            na, nb = len(a_seq), len(b_seq)
            order = [("a", x) for x in a_seq[:nbias]]
            ia, ib = nbias, 0
            if PAIRED:
                # 1:1 zip: both column groups advance at their own full rate
                # (starts are pc-monotone but execution is concurrent), so the
                # bottom group finishes at ~nb passes while A streams on
                while ib < nb:
                    order.append(("b", b_seq[ib])); ib += 1
                    if ia < na:
                        order.append(("a", a_seq[ia])); ia += 1
            else:
                order.extend(("b", x) for x in b_seq)  # bottom block first, dense
                ib = nb
            split0 = len(order)
            order.extend(("a", x) for x in a_seq[ia:])
            # phase boundaries: [0: bias+bottom][1: ~12 top passes][2: rest]
            split1 = min(split0 + 12, len(order))

            def emit_mms(phase):
                lo, hi = [(0, split0), (split0, split1), (split1, len(order))][phase]
                for grp, (out, lhsT, rhs, st, sp) in order[lo:hi]:
                    nc.tensor.matmul(out, lhsT, rhs, start=st, stop=sp,
                                     tile_position=(0, 0) if grp == "a" else bot_tpos,
                                     skip_group_check=True)

            # ---------------- activations + cell + transpose ----------------
            if PAIRED:
                ts_t = ts_b = alloc_tset("")
                c_new = statesL.tile([128, 512], F32, tag="c", name="c")
                if not (top and bottom):
                    nc.vector.memset(c_new[BOT if top else TOP, :], 0.0)
                cell_top = lambda: emit_cell(gps_t, TOP, ts_t, TOP,
                                             c_prev[TOP, :], c_new[TOP, :])
                cell_bot = lambda: emit_cell(gps_b, BOT, ts_b, BOT,
                                             c_prev[BOT, :], c_new[BOT, :])
                top_h = lambda: ts_t["hsb"][TOP, :]
                bot_h = lambda: ts_b["hsb"][BOT, :]
                bot_ident = identM[64:128, 64:128]
            else:
                ts_t = alloc_tset("t") if top else None
                ts_b = alloc_tset("b") if bottom else None
                c1_new = (states.tile([64, 512], F32, tag="c1", name="c1")
                          if top else None)
                c0_new = (states.tile([64, 512], F32, tag="c0", name="c0")
                          if bottom else None)
                cell_top = lambda: emit_cell(gps_t, slice(0, 64), ts_t, TOP,
                                             c1_prev[:, :], c1_new[:, :])
                cell_bot = lambda: emit_cell(gps_b, slice(0, 64), ts_b, TOP,
                                             c0_prev[:, :], c0_new[:, :])
                top_h = lambda: ts_t["hsb"][TOP, :]
                bot_h = lambda: ts_b["hsb"][TOP, :]
                bot_ident = identM[0:64, 0:64]

            def head():
                d = s - n_enc
                hd = tpool.tile([128, KC * B], F32, tag="hd", name="hd")[0:1, 0:B]
                nc.tensor.matmul(hd, ones64[0:1, 0:1], headb[0:1, :],
                                 start=True, stop=False)
                for k in range(KC):
                    nc.tensor.matmul(hd, headt[:, k:k + 1], h1ap(k),
                                     start=False, stop=k == KC - 1)
                if d + 1 < n_dec:
                    nc.vector.tensor_copy(dec_stage[0:1, :], hd)
                nc.vector.tensor_copy(out_acc[0:1, d * B:(d + 1) * B], hd)

            if feedback:
                # decoder: top cell -> head -> bottom x pass -> bottom cell
                emit_mms(0)
                emit_mms(1)
                emit_mms(2)
                cell_top()
                t1T = emit_transpose(top_h(), identM[0:64, 0:64], "h1T")
                h1ap = lambda k, t=t1T: t[:, k * B:(k + 1) * B]
                head()
                for j in range(NCH):
                    nc.tensor.matmul(gps_b[j][bot_gsl, :], stage[0:2, :],
                                     wbot["r0"][0:2, j * 512:(j + 1) * 512],
                                     start=first_b[j], stop=True,
                                     tile_position=bot_tpos, skip_group_check=True)
                cell_bot()
                t0T = emit_transpose(bot_h(), bot_ident, "h0T")
                h0ap = lambda k, t=t0T: t[:, k * B:(k + 1) * B]
            elif PAIRED and top and bottom:
                # full-width path: one cell chain for both cells; the o-gate /
                # h / transpose tail runs per 128-col chunk so transposes and
                # the state copies start as soon as each chunk is ready
                emit_mms(0)
                emit_mms(1)
                emit_mms(2)
                ts = ts_t
                fl = slice(0, 128)
                nc.scalar.activation(ts["ifsb"][fl, 0:512], gps_t[I_SL][fl, :], AF.Sigmoid)
                nc.scalar.activation(ts["ifsb"][fl, 512:1024], gps_t[F_SL][fl, :], AF.Sigmoid)
                nc.scalar.activation(ts["gsb"][fl, :], gps_t[G_SL][fl, :], AF.Tanh)
                nc.vector.tensor_mul(ts["t1"][fl, :], ts["ifsb"][fl, 512:1024], c_prev[:, :])
                nc.vector.tensor_mul(ts["t2"][fl, :], ts["ifsb"][fl, 0:512], ts["gsb"][fl, :])
                nc.vector.tensor_add(c_new[:, :], ts["t1"][fl, :], ts["t2"][fl, :])
                nc.scalar.activation(ts["tcsb"][fl, :], c_new[:, :], AF.Tanh)
                tps = [tpool.tile([128, KC * B], MDT, tag="tp", name=f"tpf{h2}")
                       for h2 in range(2)]
                fused = statesL.tile([128, 512], MDT, tag="hTf", name="hTf")
                for k in range(KC):
                    cs = slice(k * 128, (k + 1) * 128)
                    nc.scalar.activation(ts["osb"][fl, cs], gps_t[O_SL][fl, cs], AF.Sigmoid)
                    nc.vector.tensor_mul(ts["hsb"][fl, cs], ts["osb"][fl, cs],
                                         ts["tcsb"][fl, cs])
                    nc.tensor.transpose(tps[k // 2][:, (k % 2) * 128:(k % 2 + 1) * 128],
                                        ts["hsb"][:, cs], identM)
                    if k % 2 == 1:
                        nc.vector.tensor_copy(fused[:, (k // 2) * 256:(k // 2 + 1) * 256],
                                              tps[k // 2])
                h1ap = lambda k, t=fused: t[:, k * 128:k * 128 + 64]
                h0ap = lambda k, t=fused: t[:, k * 128 + 64:(k + 1) * 128]
            else:
                # bottom first; its transposes go into the PE stream a dozen
                # passes later so the PE reaches them just as the bottom
                # cell's ACT/DVE chain finishes (no head-of-line stall)
                emit_mms(0)
                if bottom:
                    cell_bot()
                emit_mms(1)
                if bottom:
                    t0T = emit_transpose(bot_h(), bot_ident, "h0T")
                    h0ap = lambda k, t=t0T: t[:, k * B:(k + 1) * B]
                emit_mms(2)
                if top:
                    cell_top()
                    t1T = emit_transpose(top_h(), identM[0:64, 0:64], "h1T")
                    h1ap = lambda k, t=t1T: t[:, k * B:(k + 1) * B]
                    if s >= n_enc:
                        head()

            if PAIRED:
                c_prev = c_new
            else:
                if top:
                    c1_prev = c1_new
                if bottom:
                    c0_prev = c0_new

        # prologue: layer-0 @ t=0 alone, then first fw pair unrolled
        emit_pair(-1, top=False, bottom=True)
        BODY = 8
        # encoder fw pairs s=1..n_enc-2 go through a hardware loop (8-pair
        # body); everything s-dependent inside is either constant for these
        # pairs (enc weights, no head) or overridden (x stage).
        nloop = max(0, (n_enc - 2) // BODY) if PAIRED else 0
        s_after_loop = 1 + nloop * BODY
        emit_pair(0, top=True, bottom=True)
        if nloop > 0:
            from concourse.bass import ds
            with tc.For_i(0, nloop, 1) as iv:
                stage_body = xstage.tile([2, BODY * B], MDT, tag="stage_body")
                nc.sync.dma_start(
                    out=stage_body,
                    in_=xaug[:, ds(2 * B + iv * (BODY * B), BODY * B)])
                for p in range(BODY):
                    emit_pair(1 + p, top=True, bottom=True,
                              stage_override=stage_body[:, p * B:(p + 1) * B])
        for s in range(s_after_loop, NT - 1):
            emit_pair(s, top=True, bottom=True)
        emit_pair(NT - 1, top=True, bottom=False)
        nc.sync.dma_start(out=outT[:, :], in_=out_acc)

    nc.compile()
    return nc


# ------------------------------------------------------------------
# host-side packing
# ------------------------------------------------------------------
def _np_dt(mm_dt):
    if mm_dt == "bfloat16":
        import ml_dtypes
        return ml_dtypes.bfloat16
    if mm_dt == "float16":
        return np.float16
    return np.float32


def _pack_weights(inputs, mm_dt="float32r"):
    f32 = np.float32
    ndt = _np_dt(mm_dt)

    def wt_pack(w):  # [G, H] -> [128, KC, G]
        return np.ascontiguousarray(
            np.asarray(w, f32).T.reshape(KC, 128, G).transpose(1, 0, 2)).astype(ndt)

    m = {
        "wt_e0": wt_pack(inputs["enc_Whh0"]),
        "wt_e1i": wt_pack(inputs["enc_Wih1"]),
        "wt_e1h": wt_pack(inputs["enc_Whh1"]),
        "wt_d0": wt_pack(inputs["dec_Whh0"]),
        "wt_d1i": wt_pack(inputs["dec_Wih1"]),
        "wt_d1h": wt_pack(inputs["dec_Whh1"]),
        "rows_e0": np.stack([np.asarray(inputs["enc_Wih0"], f32)[:, 0],
                             np.asarray(inputs["enc_b0"], f32)]).astype(ndt),
        "rows_e1": np.asarray(inputs["enc_b1"], f32)[None, :].astype(ndt),
        "rows_d0": np.stack([np.asarray(inputs["dec_Wih0"], f32)[:, 0],
                             np.asarray(inputs["dec_b0"], f32)]).astype(ndt),
        "rows_d1": np.asarray(inputs["dec_b1"], f32)[None, :].astype(ndt),
        "headt": np.ascontiguousarray(
            np.asarray(inputs["head_W"], f32)[0].reshape(KC, 128).T).astype(ndt),
        "headb": np.full((1, B), float(np.asarray(inputs["head_b"])[0]), ndt),
        "zeros": np.zeros((128, KC * B), ndt),
    }
    return {k: np.ascontiguousarray(v) for k, v in m.items()}


def _pack_x(xc, n_enc=T, mm_dt="float32r"):
    """xc [B, T, 1] slice -> xaug [2, (n_enc+1)*B] (row0 = x_t seq, row1 = 1)."""
    f32 = np.float32
    xt = np.asarray(xc, f32)[:, :, 0].T  # [T, B]
    xa = np.empty((2, (n_enc + 1) * B), f32)
    xa[0, :n_enc * B] = xt[:n_enc].reshape(-1)
    xa[0, n_enc * B:] = xt[T - 1]  # decoder initial input = last observed x
    xa[1, :] = 1.0
    return np.ascontiguousarray(xa.astype(_np_dt(mm_dt)))


def kernel(**inputs):
    global LAST_EXEC_NS, LAST_RESULTS
    from concourse.bass_utils import run_bass_kernel_spmd

    n_enc = int(os.environ.get("LSTM_NENC", T))
    n_dec = int(os.environ.get("LSTM_NDEC", HORIZON))
    mm_dt = os.environ.get("LSTM_MMDT", "float16")
    key = (n_enc, n_dec, mm_dt)
    if key not in _CACHE:
        _CACHE[key] = _build(n_enc, n_dec, mm_dt)
    nc = _CACHE[key]

    shared = _pack_weights(inputs, mm_dt)
    in_maps = []
    for c in range(NCORES):
        m = dict(shared)
        m["xaug"] = _pack_x(inputs["x"][c * B:(c + 1) * B], n_enc, mm_dt)
        in_maps.append(m)

    trace = os.environ.get("LSTM_TRACE", "0") == "1"
    t0 = time.time()
    res = run_bass_kernel_spmd(nc, in_maps, list(range(NCORES)), trace=trace)
    wall = time.time() - t0
    LAST_EXEC_NS = res.exec_time_ns
    LAST_RESULTS = res
    if os.environ.get("LSTM_VERBOSE", "0") == "1":
        print(f"[kernel] wall={wall:.2f}s exec_time_ns={res.exec_time_ns}", file=sys.stderr)

    out = np.empty((B_FULL, n_dec), np.float32)
    for c in range(NCORES):
        out[c * B:(c + 1) * B, :] = res.results[c]["outT"].reshape(n_dec, B).T
    return out


if __name__ == "__main__":
    cmd = sys.argv[1] if len(sys.argv) > 1 else "build"
    if cmd == "build":
        ne = int(os.environ.get("LSTM_NENC", "4"))
        nd = int(os.environ.get("LSTM_NDEC", "2"))
        md = os.environ.get("LSTM_MMDT", "float32r")
        t0 = time.time()
        nc = _build(ne, nd, md)
        print(f"build({ne},{nd},{md}) ok in {time.time()-t0:.1f}s")

